# revision 34
# baseline (speedup 1.0000x reference)
"""Trainium2 Bass kernel for MCRNNVAE eval forward (nn_MCRNNVAE_34754875359779).

Key insight: the reference network has no nonlinearity other than the RNN tanh
(PhiBlock/VariationalBlock hidden layers are linear), so per channel c the whole
per-timestep chain collapses algebraically to a vanilla RNN:

    h_{t+1} = tanh(x_t @ U_c + h_t @ M_c + ub_c)
    mu_t    = x_t @ V_c + h_t @ N_c + vb_c

with U [F,H], M [H,H], N [H,F], V [F,F] folded on the host in float64.

Sharding: 3 channels x 512 batch = 1536 recurrence columns -> 8 cores x (128+64).
Every core runs the same SPMD program with two recurrence instances (width 128
and width 64); the (channel, batch-slice) assignment is carried entirely by the
per-core input data (weights + host-transposed x slices).

On-device layout is fully transposed (features on partitions, (t, batch) on the
free axis) so every matmul uses host-shipped weights as the stationary operand
in natural layout and the tanh output lands directly in next-step layout.

Performance structure (the serial h-chain is latency-bound at
ACT(tanh) -> sem -> PE(4 M-matmuls) -> sem -> ACT, ~1us/step):
- ONE merged tanh per instance per step over the whole [128, 2, w] psum
  (ACT's fixed ~187ns psum-read latency would otherwise be paid twice).
- The tanh bias is ub0 (fp32, ACT bias operand); the j=1 half's correction
  ub1-ub0 enters via a rank-1 PE matmul that is off the serial chain.
- Instance A pads its psum so each j half owns a 2KB bank (only one psum
  accumulation group may be pending per bank): both groups open early and
  only the four h-dependent M matmuls sit on the serial chain.
- The mu path (V@x, N@h) is emitted in <=2-step pieces so its PE work hides
  inside the chain's per-step idle bubble instead of bursting every 4 steps.
- DMA issue costs ~650ns/slot and transfers serialize per queue, so inputs
  are packed into few tensors ordered by first use (hot U blob, tiny first
  x chunk, M blob, next x chunk, V/N blob, remaining x chunks) to get both
  chains started ~4.5us in; the PE clock is pre-ramped to full p-state with
  dummy matmuls during the initial DMA wait.
- Step 99's h_100 is never read -> the last rec_step is skipped entirely.
- The final two mu stores drain through the idle ACT/SP DGEs in parallel
  instead of serially through the Pool SWDGE.
"""

import os
import numpy as np

import ml_dtypes

C, T, B, F = 3, 100, 512, 128
H = 256
WA, WB = 128, 64  # per-core recurrence widths (columns of batch x channel)
WSUM = WA + WB

# (channel, b0) for each core's width-128 piece and width-64 piece.
PIECES_A = [(0, 0), (0, 128), (0, 256), (0, 384), (1, 0), (1, 128), (2, 0), (2, 128)]
PIECES_B = [(1, 256), (1, 320), (1, 384), (1, 448),
            (2, 256), (2, 320), (2, 384), (2, 448)]

BF16 = ml_dtypes.bfloat16

# Weights ship in two blobs so step 0 is gated only on the small "hot" one:
# wb_hot (bf16): per instance [0:256) U, row 0 of [256:384) ub1-ub0,
#   row 0 of [384:384+w) ones; A at column 0, B at column 512; tanh biases
#   ub0 (bf16) in columns 1024 (A) and 1025 (B).
# wb_m (bf16): per instance [0:256) M0, [256:512) M1; A at column 0,
#   B at column 512 (needed from step 1).
# wb_vn (bf16): per instance [0:128) V, [128:256) N0, [256:384) N1;
#   A at 0, B at 384 (first needed by the t=2 mu pieces).
HOTB = 512          # hot-blob stride per instance
BHOT = 2 * HOTB + 2  # hot blob total columns (incl. 2 bias columns)
MB = 512            # M-blob stride per instance
BM = 2 * MB
VNB = 384           # V/N-blob stride per instance
BVN = 2 * VNB


def _fold_weights(inputs):
    """Collapse the linear chain per channel, in float64. Returns per-channel
    (U [128,256], ub [256], M [256,256], N [256,128], V [128,128], vb [128])."""
    HX, HZ, EH, L = 128, 128, 128, 64
    g = lambda k: np.asarray(inputs[k], np.float64)
    out = []
    for c in range(C):
        Wx, bx = g("phi_x_W")[c], g("phi_x_b")[c]
        We, be = g("enc_W")[c], g("enc_b")[c]
        Wqm, bqm = g("enc_mu_W")[c], g("enc_mu_b")[c]
        Wz, bz = g("phi_z_W"), g("phi_z_b")
        Wd, bd = g("dec_W")[c], g("dec_b")[c]
        Wpm, bpm = g("dec_mu_W")[c], g("dec_mu_b")[c]
        Wih, Whh = g("rnn_Wih"), g("rnn_Whh")
        bih, bhh = g("rnn_bih"), g("rnn_bhh")

        We_x, We_h = We[:HX], We[HX:]
        Wd_z, Wd_h = Wd[:HZ], Wd[HZ:]
        Wih_x, Wih_z = Wih[:HX], Wih[HX:]

        PWz = Wqm @ Wz                     # [EH, HZ]
        P = We_x @ PWz                     # [HX, HZ]
        Q = We_h @ PWz                     # [H, HZ]
        r = be @ PWz + bqm @ Wz + bz       # [HZ]

        G = Wih_x + P @ Wih_z              # [HX, H]
        M = Q @ Wih_z + Whh                # [H, H]
        gv = r @ Wih_z + bih + bhh         # [H]

        U = Wx @ G                         # [F, H]
        ub = bx @ G + gv                   # [H]

        W2 = Wd_z @ Wpm                    # [HZ, F]
        V = Wx @ (P @ W2)                  # [F, F]
        N = (Q @ Wd_z + Wd_h) @ Wpm        # [H, F]
        vb = bx @ (P @ W2) + r @ W2 + bd @ Wpm + bpm  # [F]
        out.append((U, ub, M, N, V, vb))
    return out


_NC_CACHE = {}

XCHUNKS = [(0, 2), (2, 8), (8, 25), (25, 50), (50, 75), (75, 100)]


def _build_nc():
    if "nc" in _NC_CACHE:
        return _NC_CACHE["nc"]
    import concourse.bacc as bacc
    import concourse.mybir as mybir
    import concourse.tile as tile

    DT = mybir.dt.bfloat16
    F32 = mybir.dt.float32
    Tanh = mybir.ActivationFunctionType.Tanh

    nc = bacc.Bacc()

    dram = {
        "xT": nc.declare_dram_parameter("xT", [128, T, WSUM], DT, isOutput=False),
        "wbh": nc.declare_dram_parameter("wbh", [128, BHOT], DT, isOutput=False),
        "wm": nc.declare_dram_parameter("wm", [128, BM], DT, isOutput=False),
        "wvn": nc.declare_dram_parameter("wvn", [128, BVN], DT, isOutput=False),
        "out_a": nc.declare_dram_parameter("out_a", [128, T, WA], DT, isOutput=True),
        "out_b": nc.declare_dram_parameter("out_b", [128, T, WB], DT, isOutput=True),
    }

    with tile.TileContext(nc) as tc:
        with (
            tc.tile_pool(name="wts", bufs=1) as wpool,
            tc.tile_pool(name="big", bufs=1) as xpool,
            tc.tile_pool(name="mu_out", bufs=1) as mupool,
            tc.tile_pool(name="ps_a", bufs=2, space="PSUM") as ps_a,
            tc.tile_pool(name="ps_b", bufs=1, space="PSUM") as ps_b,
            tc.tile_pool(name="mu_a", bufs=1, space="PSUM") as mu_a,
            tc.tile_pool(name="mu_b", bufs=1, space="PSUM") as mu_b,
            tc.tile_pool(name="ps_scr", bufs=1, space="PSUM") as ps_scr,
        ):
            inst = {}
            scr = ps_scr.tile([1, 512], mybir.dt.float32, tag="scr", name="scr")
            gscr = wpool.tile([128, 1], mybir.dt.bfloat16, tag="gscr", name="gscr")

            # PE p-state pre-ramp: the tensor engine clock reaches full speed
            # only after ~3us of continuous execution. Grind wide dummy
            # matmuls while the first DMAs stream so the early recurrence
            # steps already run at the full clock.
            dum = wpool.tile([1, 512], DT, tag="dum", name="dum")
            nc.gpsimd.memset(dum[:], 0.0)
            for _ in range(7):
                nc.tensor.matmul(scr[:], dum[0:1, 0:1], dum[:],
                                 start=True, stop=True)

            wbh = wpool.tile([128, BHOT], DT, tag="wbh", name="wbh")
            nc.sync.dma_start(wbh[:], dram["wbh"][:])
            # Prime PE and ACT on the hot-blob DMA once (dummy ops), so real
            # matmuls/tanhs never need a second (DMA) sync wait.
            nc.tensor.matmul(scr[:, 0:1], wbh[:, 0:1], wbh[:, 0:1],
                             start=True, stop=True)
            warm = wpool.tile([128, 2], DT, tag="warm", name="warm")
            nc.scalar.activation(warm[:], wbh[:, 1024:1026], Tanh)
            # DMA issue order on the SP queue = step-0 gating order: the
            # first x chunk right after the hot blob, the rest blob (first
            # used at step 1) after it, then the remaining x chunks.
            xT = xpool.tile([128, T, WSUM], DT, tag="xT", name="xT")
            s0, e0 = XCHUNKS[0]
            nc.sync.dma_start(xT[:, s0:e0, :], dram["xT"][:, s0:e0, :])
            wm = wpool.tile([128, BM], DT, tag="wm", name="wm")
            nc.sync.dma_start(wm[:], dram["wm"][:])
            s1, e1 = XCHUNKS[1]
            nc.sync.dma_start(xT[:, s1:e1, :], dram["xT"][:, s1:e1, :])
            wvn = wpool.tile([128, BVN], DT, tag="wvn", name="wvn")
            nc.sync.dma_start(wvn[:], dram["wvn"][:])
            for s, e in XCHUNKS[2:]:
                nc.sync.dma_start(xT[:, s:e, :], dram["xT"][:, s:e, :])

            for sfx, w, off, pspool, mupsp in (
                    ("a", WA, 0, ps_a, mu_a),
                    ("b", WB, WA, ps_b, mu_b)):
                d = {}
                hot = wbh[:, (0 if sfx == "a" else HOTB):]
                mm = wm[:, (0 if sfx == "a" else MB):]
                vn = wvn[:, (0 if sfx == "a" else VNB):]
                d["U"] = hot[:, 0:256]
                d["dub"] = hot[0:1, 256:384]   # ub1 - ub0 row
                d["ones"] = hot[0:1, 384:384 + w]
                d["M0"] = mm[:, 0:256]
                d["M1"] = mm[:, 256:512]
                d["V"] = vn[:, 0:128]
                d["N0"] = vn[:, 128:256]
                d["N1"] = vn[:, 256:384]
                d["bias"] = wbh[:, 1024:1025] if sfx == "a" else wbh[:, 1025:1026]
                d["x"] = lambda lo, hi, off=off, w=w: xT[:, lo:hi, off:off + w]
                # h state history: block t holds h_t (transposed, bf16) as
                # [2, w] (features 0:128 then 128:256). Block 0 (h_0 = 0) is
                # never touched: step 0 and the mu pieces special-case it.
                d["h"] = xpool.tile([128, T + 1, 2, w], DT, tag=f"h{sfx}",
                                    name=f"h{sfx}")
                d["w"] = w
                d["pspool"] = pspool
                d["mupool"] = mupsp
                d["mu_ps"] = None
                d["prev_ot"] = None
                inst[sfx] = d

            def rec_step(sfx, t):
                d = inst[sfx]
                w = d["w"]
                # Only one accumulation group may be pending per 2KB psum
                # bank. Instance A pads its psum tile so each j half gets its
                # own bank: both groups open early (U/dub run during the
                # previous step's tanh) and only the four h-dependent M
                # matmuls sit on the serial chain. Instance B (more slack)
                # keeps one bank with sequential j groups and bufs=1.
                split = sfx == "a"
                pw = 512 if split else w
                ps = d["pspool"].tile([128, 2, pw], mybir.dt.float32,
                                      tag=f"ps{sfx}", name=f"ps{sfx}")
                first = t == 0
                U, M0, M1 = d["U"], d["M0"], d["M1"]
                xt = d["x"](t, t + 1)[:, 0, :]

                def mj(j, stat, mov, start, stop):
                    nc.tensor.matmul(ps[:, j, 0:w], stat, mov,
                                     start=start, stop=stop)

                if split:
                    mj(0, U[:, 0:128], xt, True, first)
                    mj(1, d["dub"], d["ones"], True, False)
                    mj(1, U[:, 128:256], xt, False, first)
                    if not first:
                        for j in (0, 1):
                            mj(j, M0[:, j * 128:(j + 1) * 128],
                               d["h"][:, t, 0, :], False, False)
                            mj(j, M1[:, j * 128:(j + 1) * 128],
                               d["h"][:, t, 1, :], False, True)
                else:
                    mj(0, U[:, 0:128], xt, True, first)
                    if not first:
                        mj(0, M0[:, 0:128], d["h"][:, t, 0, :], False, False)
                        mj(0, M1[:, 0:128], d["h"][:, t, 1, :], False, True)
                    mj(1, d["dub"], d["ones"], True, False)
                    mj(1, U[:, 128:256], xt, False, first)
                    if not first:
                        mj(1, M0[:, 128:256], d["h"][:, t, 0, :], False, False)
                        mj(1, M1[:, 128:256], d["h"][:, t, 1, :], False, True)
                nc.scalar.activation(d["h"][:, t + 1, :, :], ps[:, :, 0:w],
                                     Tanh, bias=d["bias"])

            # --- mu path, emitted in small pieces -------------------------
            def mu_open(sfx, t0, nt):
                d = inst[sfx]
                w = d["w"]
                # Primer: observe the previous chunk's DVE copy on PE so the
                # V-matmul's psum-slot WAR needs only the PE-self wait. The
                # dummy matmul targets this chunk's own (currently closed)
                # bank.
                full = 4 if sfx == "a" else 8
                ps = d["mupool"].tile([128, full, w], mybir.dt.float32,
                                      tag=f"mu{sfx}", name=f"mups{sfx}")
                prev = d["prev_ot"]
                if prev is not None:
                    nc.tensor.matmul(scr[:, 0:1], prev[:, 0, 0:1],
                                     prev[:, 0, 0:1], start=True, stop=True)
                nc.tensor.matmul(ps[:, 0:nt, :], d["V"], d["x"](t0, t0 + nt),
                                 start=True, stop=False)
                d["mu_ps"] = ps
                d["mu_t0"] = t0
                d["mu_nt"] = nt

            def mu_n(sfx, lo, hi, stop):
                d = inst[sfx]
                t0 = d["mu_t0"]
                # h_0 = 0 and block 0 of h is uninitialized: skip time 0
                # (mu_0 = V@x_0 + vb exactly).
                lo = max(lo, 1)
                ps = d["mu_ps"]
                nc.tensor.matmul(ps[:, lo - t0:hi - t0, :], d["N0"],
                                 d["h"][:, lo:hi, 0, :], start=False, stop=False)
                nc.tensor.matmul(ps[:, lo - t0:hi - t0, :], d["N1"],
                                 d["h"][:, lo:hi, 1, :], start=False, stop=stop)

            def mu_close(sfx, idx, final=False):
                d = inst[sfx]
                w = d["w"]
                t0, nt = d["mu_t0"], d["mu_nt"]
                # DVE copy to a never-reused bf16 staging tile (no WAR -> the
                # copy's single wait is the PE psum dep), then GPSIMD-issued
                # (SWDGE, multi-wait-capable) DMA out. vb is added on the host.
                ot = mupool.tile([128, nt, w], DT, tag=f"mu{sfx}{idx}",
                                 name=f"mu{sfx}{idx}")
                nc.vector.tensor_copy(ot[:], d["mu_ps"][:, 0:nt, :])
                if final:
                    # Drain path: the ACT and SP DGEs are idle at this point;
                    # issuing the two last stores there runs them in parallel
                    # instead of serially through the Pool SWDGE.
                    eng = nc.scalar if sfx == "a" else nc.sync
                    eng.dma_start(dram[f"out_{sfx}"][:, t0:t0 + nt, :], ot[:])
                else:
                    # GPSIMD interposer: observe the DVE copy in GPSIMD
                    # program order so the out-DMA instruction needs only its
                    # queue wait.
                    nc.gpsimd.tensor_copy(gscr[:], ot[:, 0, 0:1])
                    nc.gpsimd.dma_start(dram[f"out_{sfx}"][:, t0:t0 + nt, :],
                                        ot[:])
                d["prev_ot"] = ot

            def mu_pieces(t):
                # A: chunks of 4 steps; V at t%4==1, N halves at 2 and 3.
                # The last 4-step chunk (t0=96) is split into two 2-step
                # chunks so the post-last-tanh store chain is short.
                if t < 96:
                    p = t % 4
                    if p == 1 and t > 1:
                        mu_open("a", t - 1, 4)
                    elif p == 2:
                        if t == 2:
                            mu_open("a", 0, 4)
                        mu_n("a", t - 2, t, False)
                    elif p == 3:
                        mu_n("a", t - 1, t + 1, True)
                        mu_close("a", t // 4)
                elif t == 97:
                    mu_open("a", 96, 2)
                    mu_n("a", 96, 98, True)
                    mu_close("a", 24)
                elif t == 98:
                    mu_open("a", 98, 2)
                elif t == 99:
                    mu_n("a", 98, 100, True)
                    mu_close("a", 25, final=True)
                # B: chunks of 8 steps; V at t%8==2, N pairs at 4,5,6,7.
                # Tail (t0=96, 4 steps): V at 97, N pairs at 98, 99.
                if t < 96:
                    q = t % 8
                    if q == 2:
                        mu_open("b", t - 2, 8)
                    elif q == 4:
                        mu_n("b", t - 4, t - 2, False)
                    elif q == 5:
                        mu_n("b", t - 3, t - 1, False)
                    elif q == 6:
                        mu_n("b", t - 2, t, False)
                    elif q == 7:
                        mu_n("b", t - 1, t + 1, True)
                        mu_close("b", t // 8)
                elif t == 97:
                    mu_open("b", 96, 4)
                elif t == 98:
                    mu_n("b", 96, 98, False)
                elif t == 99:
                    mu_n("b", 98, 100, True)
                    mu_close("b", 12, final=True)

            chunk_starts = {s: e for s, e in XCHUNKS}
            for t in range(T):
                if t in chunk_starts:
                    # Primer: let PE observe the xT chunk DMA once, so real
                    # matmuls never need a second (DMA) wait.
                    e = chunk_starts[t]
                    nc.tensor.matmul(scr[:, 0:1], xT[:, e - 1, 0:1],
                                     xT[:, e - 1, 0:1], start=True, stop=True)
                if t == 1:
                    # Prime PE on the M-blob DMA, first used here.
                    nc.tensor.matmul(scr[:, 0:1], wm[:, 0:1], wm[:, 0:1],
                                     start=True, stop=True)
                if t < T - 1:
                    # Step 99 would only produce h_100, which nothing reads.
                    rec_step("a", t)
                    rec_step("b", t)
                if t == 2:
                    # Prime PE on the V/N-blob DMA, first used by the mu
                    # pieces below (after the chain's M matmuls, so a late
                    # DMA never stalls the chain).
                    nc.tensor.matmul(scr[:, 0:1], wvn[:, 0:1], wvn[:, 0:1],
                                     start=True, stop=True)
                mu_pieces(t)

    nc.compile()
    _NC_CACHE["nc"] = nc
    return nc


def _prepare_in_maps(inputs):
    folded = _fold_weights(inputs)
    x = np.asarray(inputs["x"], np.float32)

    def hot_cols(c, w):
        U, ub, M, N, V, vb = folded[c]
        blob = np.zeros((128, HOTB), np.float32)
        blob[:, 0:256] = U
        blob[0, 256:384] = ub[128:] - ub[:128]
        blob[0, 384:384 + w] = 1.0
        return blob

    def m_cols(c):
        U, ub, M, N, V, vb = folded[c]
        return np.concatenate([M[:128], M[128:]], axis=1)

    def vn_cols(c):
        U, ub, M, N, V, vb = folded[c]
        return np.concatenate([V, N[:128], N[128:]], axis=1)

    in_maps = []
    for core in range(8):
        ca, ba = PIECES_A[core]
        cb, bb = PIECES_B[core]
        xa = x[ca, :, ba:ba + WA, :].transpose(2, 0, 1)   # [128, T, WA]
        xb = x[cb, :, bb:bb + WB, :].transpose(2, 0, 1)   # [128, T, WB]
        xall = np.concatenate([xa, xb], axis=2)           # [128, T, 192]
        wbh = np.zeros((128, BHOT), np.float32)
        wbh[:, :2 * HOTB] = np.concatenate(
            [hot_cols(ca, WA), hot_cols(cb, WB)], axis=1)
        wm = np.concatenate([m_cols(ca), m_cols(cb)], axis=1)
        wvn = np.concatenate([vn_cols(ca), vn_cols(cb)], axis=1)
        wbh[:, 2 * HOTB] = folded[ca][1][:128]
        wbh[:, 2 * HOTB + 1] = folded[cb][1][:128]
        in_maps.append({
            "xT": np.ascontiguousarray(xall).astype(BF16),
            "wbh": wbh.astype(BF16),
            "wm": wm.astype(np.float32).astype(BF16),
            "wvn": wvn.astype(np.float32).astype(BF16),
        })
    return in_maps, folded


def kernel(**inputs):
    from concourse.bass_utils import run_bass_kernel_spmd

    in_maps, folded = _prepare_in_maps(inputs)
    nc = _build_nc()
    trace = bool(int(os.environ.get("BASS_KERNEL_TRACE", "0")))
    res = run_bass_kernel_spmd(nc, in_maps, core_ids=list(range(8)),
                               trace=trace)
    kernel.last_results = res

    out = np.empty((C, T, B, F), np.float32)
    for core in range(8):
        for sfx, w, (c, b0) in (("a", WA, PIECES_A[core]),
                                ("b", WB, PIECES_B[core])):
            muT = np.asarray(res.results[core][f"out_{sfx}"]).astype(np.float32)
            vb = folded[c][5].astype(np.float32)
            out[c, :, b0:b0 + w, :] = muT.reshape(128, T, w).transpose(1, 2, 0) + vb
    return out


# revision 40
# speedup vs baseline: 1.0461x; 1.0461x over previous
"""Trainium2 Bass kernel for MCRNNVAE eval forward (nn_MCRNNVAE_34754875359779).

Key insight: the reference network has no nonlinearity other than the RNN tanh
(PhiBlock/VariationalBlock hidden layers are linear), so per channel c the whole
per-timestep chain collapses algebraically to a vanilla RNN:

    h_{t+1} = tanh(x_t @ U_c + h_t @ M_c + ub_c)
    mu_t    = x_t @ V_c + h_t @ N_c + vb_c

with U [F,H], M [H,H], N [H,F], V [F,F] folded on the host in float64.

Sharding: 3 channels x 512 batch = 1536 recurrence columns -> 8 cores x
(108+86). Chains may cover overlapping batch slices (512 doesn't divide into
108/86 evenly; duplicated columns compute identical values and the host
assembly overwrites them). Every core runs the same SPMD program with two
recurrence instances; the (channel, batch-slice) assignment is carried
entirely by the per-core input data (weights + host-transposed x slices).
The width pair balances the two serial chains: the wide chain is
psum-bank-split (U/bias off-chain, ~952ns/step), the narrow one keeps
sequential j-groups in one bank whose longer on-chain matmul segment
(~944ns/step) still fits under the wide chain's period.

On-device layout is fully transposed (features on partitions, (t, batch) on the
free axis) so every matmul uses host-shipped weights as the stationary operand
in natural layout and the tanh output lands directly in next-step layout.

Performance structure (the serial h-chain is latency-bound at
ACT(tanh) -> sem -> PE(4 M-matmuls) -> sem -> ACT, ~1us/step):
- ONE merged tanh per instance per step over the whole [128, 2, w] psum
  (ACT's fixed ~187ns psum-read latency would otherwise be paid twice).
- The tanh bias is ub0 (fp32, ACT bias operand); the j=1 half's correction
  ub1-ub0 enters via a rank-1 PE matmul that is off the serial chain.
- Instance A pads its psum so each j half owns a 2KB bank (only one psum
  accumulation group may be pending per bank): both groups open early and
  only the four h-dependent M matmuls sit on the serial chain. Freeing the
  scratch bank for this (primers dropped; Tile's per-engine sem scoreboard
  absorbs first-reader DMA waits) lets both instances double-buffer.
- The mu path (V@x, N@h) is emitted in <=2-step pieces so its PE work hides
  inside the chain's per-step idle bubble instead of bursting every 4 steps.
- DMA issue costs ~650ns/slot and transfers serialize per queue, so inputs
  are packed into few tensors ordered by first use (hot U blob, tiny first
  x chunk, M blob, next x chunk, V/N blob, remaining x chunks) to get both
  chains started ~4.5us in; the PE clock is pre-ramped to full p-state with
  dummy matmuls during the initial DMA wait.
- Step 99's h_100 is never read -> the last rec_step is skipped entirely.
- The final two mu stores drain through the idle ACT/SP DGEs in parallel
  instead of serially through the Pool SWDGE.
"""

import os
import numpy as np

import ml_dtypes

C, T, B, F = 3, 100, 512, 128
H = 256
WA, WB = 108, 86  # per-core recurrence widths (columns of batch x channel)
WSUM = WA + WB

# (channel, b0) for each core's width-108 piece and width-86 piece.
# Chains may overlap in batch columns (512 doesn't divide evenly); the host
# assembly just overwrites the duplicated columns with identical values.
PIECES_A = [(0, 0), (0, 108), (0, 216), (0, 324),
            (1, 0), (1, 108), (1, 216), (1, 324)]
PIECES_B = [(0, 426), (1, 426), (2, 0), (2, 86),
            (2, 172), (2, 258), (2, 344), (2, 426)]

BF16 = ml_dtypes.bfloat16

# Weights ship in two blobs so step 0 is gated only on the small "hot" one:
# wb_hot (bf16): per instance [0:256) U, row 0 of [256:384) ub1-ub0,
#   row 0 of [384:384+w) ones; A at column 0, B at column 512; tanh biases
#   ub0 (bf16) in columns 1024 (A) and 1025 (B).
# wb_m (bf16): per instance [0:256) M0, [256:512) M1; A at column 0,
#   B at column 512 (needed from step 1).
# wb_vn (bf16): per instance [0:128) V, [128:256) N0, [256:384) N1;
#   A at 0, B at 384 (first needed by the t=2 mu pieces).
HOTB = 512          # hot-blob stride per instance
BHOT = 2 * HOTB + 2  # hot blob total columns (incl. 2 bias columns)
MB = 512            # M-blob stride per instance
BM = 2 * MB
VNB = 384           # V/N-blob stride per instance
BVN = 2 * VNB


def _fold_weights(inputs):
    """Collapse the linear chain per channel, in float64. Returns per-channel
    (U [128,256], ub [256], M [256,256], N [256,128], V [128,128], vb [128])."""
    HX, HZ, EH, L = 128, 128, 128, 64
    g = lambda k: np.asarray(inputs[k], np.float64)
    out = []
    for c in range(C):
        Wx, bx = g("phi_x_W")[c], g("phi_x_b")[c]
        We, be = g("enc_W")[c], g("enc_b")[c]
        Wqm, bqm = g("enc_mu_W")[c], g("enc_mu_b")[c]
        Wz, bz = g("phi_z_W"), g("phi_z_b")
        Wd, bd = g("dec_W")[c], g("dec_b")[c]
        Wpm, bpm = g("dec_mu_W")[c], g("dec_mu_b")[c]
        Wih, Whh = g("rnn_Wih"), g("rnn_Whh")
        bih, bhh = g("rnn_bih"), g("rnn_bhh")

        We_x, We_h = We[:HX], We[HX:]
        Wd_z, Wd_h = Wd[:HZ], Wd[HZ:]
        Wih_x, Wih_z = Wih[:HX], Wih[HX:]

        PWz = Wqm @ Wz                     # [EH, HZ]
        P = We_x @ PWz                     # [HX, HZ]
        Q = We_h @ PWz                     # [H, HZ]
        r = be @ PWz + bqm @ Wz + bz       # [HZ]

        G = Wih_x + P @ Wih_z              # [HX, H]
        M = Q @ Wih_z + Whh                # [H, H]
        gv = r @ Wih_z + bih + bhh         # [H]

        U = Wx @ G                         # [F, H]
        ub = bx @ G + gv                   # [H]

        W2 = Wd_z @ Wpm                    # [HZ, F]
        V = Wx @ (P @ W2)                  # [F, F]
        N = (Q @ Wd_z + Wd_h) @ Wpm        # [H, F]
        vb = bx @ (P @ W2) + r @ W2 + bd @ Wpm + bpm  # [F]
        out.append((U, ub, M, N, V, vb))
    return out


_NC_CACHE = {}

XCHUNKS = [(0, 2), (2, 8), (8, 25), (25, 50), (50, 75), (75, 100)]


def _build_nc():
    if "nc" in _NC_CACHE:
        return _NC_CACHE["nc"]
    import concourse.bacc as bacc
    import concourse.mybir as mybir
    import concourse.tile as tile

    DT = mybir.dt.bfloat16
    F32 = mybir.dt.float32
    Tanh = mybir.ActivationFunctionType.Tanh

    nc = bacc.Bacc()

    dram = {
        "xT": nc.declare_dram_parameter("xT", [128, T, WSUM], DT, isOutput=False),
        "wbh": nc.declare_dram_parameter("wbh", [128, BHOT], DT, isOutput=False),
        "wm": nc.declare_dram_parameter("wm", [128, BM], DT, isOutput=False),
        "wvn": nc.declare_dram_parameter("wvn", [128, BVN], DT, isOutput=False),
        "out_a": nc.declare_dram_parameter("out_a", [128, T, WA], DT, isOutput=True),
        "out_b": nc.declare_dram_parameter("out_b", [128, T, WB], DT, isOutput=True),
    }

    with tile.TileContext(nc) as tc:
        with (
            tc.tile_pool(name="wts", bufs=1) as wpool,
            tc.tile_pool(name="big", bufs=1) as xpool,
            tc.tile_pool(name="mu_out", bufs=1) as mupool,
            tc.tile_pool(name="ps_a", bufs=2, space="PSUM") as ps_a,
            tc.tile_pool(name="ps_b", bufs=2, space="PSUM") as ps_b,
            tc.tile_pool(name="mu_a", bufs=1, space="PSUM") as mu_a,
            tc.tile_pool(name="mu_b", bufs=1, space="PSUM") as mu_b,
        ):
            inst = {}
            gscr = wpool.tile([128, 1], mybir.dt.bfloat16, tag="gscr", name="gscr")

            # PE p-state pre-ramp: the tensor engine clock reaches full speed
            # only after ~3us of continuous execution. Grind wide dummy
            # matmuls while the first DMAs stream so the early recurrence
            # steps already run at the full clock. They borrow instance A's
            # mu psum bank (same tag -> same buffer), closed before its
            # first real chunk opens.
            dum = wpool.tile([1, 512], DT, tag="dum", name="dum")
            nc.gpsimd.memset(dum[:], 0.0)
            dumps = mu_a.tile([1, 512], mybir.dt.float32, tag="mua",
                              name="dumps")
            for _ in range(7):
                nc.tensor.matmul(dumps[:], dum[0:1, 0:1], dum[:],
                                 start=True, stop=True)

            wbh = wpool.tile([128, BHOT], DT, tag="wbh", name="wbh")
            nc.sync.dma_start(wbh[:], dram["wbh"][:])
            # Prime ACT on the hot-blob DMA once (dummy tanh), so real tanhs
            # never need a second (DMA) sync wait for the bias operand.
            warm = wpool.tile([128, 2], DT, tag="warm", name="warm")
            nc.scalar.activation(warm[:], wbh[:, 1024:1026], Tanh)
            # DMA issue order on the SP queue = step-0 gating order: the
            # first x chunk right after the hot blob, the rest blob (first
            # used at step 1) after it, then the remaining x chunks.
            xT = xpool.tile([128, T, WSUM], DT, tag="xT", name="xT")
            s0, e0 = XCHUNKS[0]
            nc.sync.dma_start(xT[:, s0:e0, :], dram["xT"][:, s0:e0, :])
            wm = wpool.tile([128, BM], DT, tag="wm", name="wm")
            nc.sync.dma_start(wm[:], dram["wm"][:])
            s1, e1 = XCHUNKS[1]
            nc.sync.dma_start(xT[:, s1:e1, :], dram["xT"][:, s1:e1, :])
            wvn = wpool.tile([128, BVN], DT, tag="wvn", name="wvn")
            nc.sync.dma_start(wvn[:], dram["wvn"][:])
            for s, e in XCHUNKS[2:]:
                nc.sync.dma_start(xT[:, s:e, :], dram["xT"][:, s:e, :])

            for sfx, w, off, pspool, mupsp in (
                    ("a", WA, 0, ps_a, mu_a),
                    ("b", WB, WA, ps_b, mu_b)):
                d = {}
                hot = wbh[:, (0 if sfx == "a" else HOTB):]
                mm = wm[:, (0 if sfx == "a" else MB):]
                vn = wvn[:, (0 if sfx == "a" else VNB):]
                d["U"] = hot[:, 0:256]
                d["dub"] = hot[0:1, 256:384]   # ub1 - ub0 row
                d["ones"] = hot[0:1, 384:384 + w]
                d["M0"] = mm[:, 0:256]
                d["M1"] = mm[:, 256:512]
                d["V"] = vn[:, 0:128]
                d["N0"] = vn[:, 128:256]
                d["N1"] = vn[:, 256:384]
                d["bias"] = wbh[:, 1024:1025] if sfx == "a" else wbh[:, 1025:1026]
                d["x"] = lambda lo, hi, off=off, w=w: xT[:, lo:hi, off:off + w]
                # h state history: block t holds h_t (transposed, bf16) as
                # [2, w] (features 0:128 then 128:256). Block 0 (h_0 = 0) is
                # never touched: step 0 and the mu pieces special-case it.
                d["h"] = xpool.tile([128, T + 1, 2, w], DT, tag=f"h{sfx}",
                                    name=f"h{sfx}")
                d["w"] = w
                d["pspool"] = pspool
                d["mupool"] = mupsp
                d["mu_ps"] = None
                d["prev_ot"] = None
                inst[sfx] = d

            def rec_step(sfx, t):
                d = inst[sfx]
                w = d["w"]
                # Only one accumulation group may be pending per 2KB psum
                # bank. Instance A pads its psum tile so each j half gets its
                # own bank: both groups open early (U/dub run during the
                # previous step's tanh) and only the four h-dependent M
                # matmuls sit on the serial chain. Instance B (more slack)
                # keeps one bank with sequential j groups and bufs=1.
                split = sfx == "a"
                pw = 512 if split else w
                ps = d["pspool"].tile([128, 2, pw], mybir.dt.float32,
                                      tag=f"ps{sfx}", name=f"ps{sfx}")
                first = t == 0
                U, M0, M1 = d["U"], d["M0"], d["M1"]
                xt = d["x"](t, t + 1)[:, 0, :]

                def mj(j, stat, mov, start, stop):
                    nc.tensor.matmul(ps[:, j, 0:w], stat, mov,
                                     start=start, stop=stop)

                if split:
                    mj(0, U[:, 0:128], xt, True, first)
                    mj(1, d["dub"], d["ones"], True, False)
                    mj(1, U[:, 128:256], xt, False, first)
                    if not first:
                        for j in (0, 1):
                            mj(j, M0[:, j * 128:(j + 1) * 128],
                               d["h"][:, t, 0, :], False, False)
                            mj(j, M1[:, j * 128:(j + 1) * 128],
                               d["h"][:, t, 1, :], False, True)
                else:
                    mj(0, U[:, 0:128], xt, True, first)
                    if not first:
                        mj(0, M0[:, 0:128], d["h"][:, t, 0, :], False, False)
                        mj(0, M1[:, 0:128], d["h"][:, t, 1, :], False, True)
                    mj(1, d["dub"], d["ones"], True, False)
                    mj(1, U[:, 128:256], xt, False, first)
                    if not first:
                        mj(1, M0[:, 128:256], d["h"][:, t, 0, :], False, False)
                        mj(1, M1[:, 128:256], d["h"][:, t, 1, :], False, True)
                nc.scalar.activation(d["h"][:, t + 1, :, :], ps[:, :, 0:w],
                                     Tanh, bias=d["bias"])

            # --- mu path, emitted in small pieces -------------------------
            def mu_open(sfx, t0, nt):
                d = inst[sfx]
                w = d["w"]
                # Primer: observe the previous chunk's DVE copy on PE so the
                # V-matmul's psum-slot WAR needs only the PE-self wait. The
                # dummy matmul targets this chunk's own (currently closed)
                # bank.
                tag = "mua" if sfx == "a" else "mub"
                ps = d["mupool"].tile([128, 4, w], mybir.dt.float32,
                                      tag=tag, name=f"mups{sfx}")
                prev = d["prev_ot"]
                if prev is not None:
                    nc.tensor.matmul(ps[0:1, 0, 0:1], prev[:, 0, 0:1],
                                     prev[:, 0, 0:1], start=True, stop=True)
                nc.tensor.matmul(ps[:, 0:nt, :], d["V"], d["x"](t0, t0 + nt),
                                 start=True, stop=False)
                d["mu_ps"] = ps
                d["mu_t0"] = t0
                d["mu_nt"] = nt

            def mu_n(sfx, lo, hi, stop):
                d = inst[sfx]
                t0 = d["mu_t0"]
                # h_0 = 0 and block 0 of h is uninitialized: skip time 0
                # (mu_0 = V@x_0 + vb exactly).
                lo = max(lo, 1)
                ps = d["mu_ps"]
                nc.tensor.matmul(ps[:, lo - t0:hi - t0, :], d["N0"],
                                 d["h"][:, lo:hi, 0, :], start=False, stop=False)
                nc.tensor.matmul(ps[:, lo - t0:hi - t0, :], d["N1"],
                                 d["h"][:, lo:hi, 1, :], start=False, stop=stop)

            def mu_close(sfx, idx, final=False):
                d = inst[sfx]
                w = d["w"]
                t0, nt = d["mu_t0"], d["mu_nt"]
                # DVE copy to a never-reused bf16 staging tile (no WAR -> the
                # copy's single wait is the PE psum dep), then GPSIMD-issued
                # (SWDGE, multi-wait-capable) DMA out. vb is added on the host.
                ot = mupool.tile([128, nt, w], DT, tag=f"mu{sfx}{idx}",
                                 name=f"mu{sfx}{idx}")
                nc.vector.tensor_copy(ot[:], d["mu_ps"][:, 0:nt, :])
                if final:
                    # Drain path: the ACT and SP DGEs are idle at this point;
                    # issuing the two last stores there runs them in parallel
                    # instead of serially through the Pool SWDGE.
                    eng = nc.scalar if sfx == "a" else nc.sync
                    eng.dma_start(dram[f"out_{sfx}"][:, t0:t0 + nt, :], ot[:])
                else:
                    # GPSIMD interposer: observe the DVE copy in GPSIMD
                    # program order so the out-DMA instruction needs only its
                    # queue wait.
                    nc.gpsimd.tensor_copy(gscr[:], ot[:, 0, 0:1])
                    nc.gpsimd.dma_start(dram[f"out_{sfx}"][:, t0:t0 + nt, :],
                                        ot[:])
                d["prev_ot"] = ot

            def mu_pieces(t):
                # A: chunks of 4 steps; V at t%4==1, N halves at 2 and 3.
                # The last 4-step chunk (t0=96) is split into two 2-step
                # chunks so the post-last-tanh store chain is short.
                if t < 96:
                    p = t % 4
                    if p == 1 and t > 1:
                        mu_open("a", t - 1, 4)
                    elif p == 2:
                        if t == 2:
                            mu_open("a", 0, 4)
                        mu_n("a", t - 2, t, False)
                    elif p == 3:
                        mu_n("a", t - 1, t + 1, True)
                        mu_close("a", t // 4)
                elif t == 97:
                    mu_open("a", 96, 2)
                    mu_n("a", 96, 98, True)
                    mu_close("a", 24)
                elif t == 98:
                    mu_open("a", 98, 2)
                elif t == 99:
                    mu_n("a", 98, 100, True)
                    mu_close("a", 25, final=True)
                # B: 4-step chunks staggered one step after A's: V at 4k+2,
                # N[4k,4k+2) at 4k+3, N[4k+2,4k+4)+close at 4k+4.
                # Tail chunk (t0=96): V at 98, both N pairs + close at 99.
                if t % 4 == 2 and t < 98:
                    mu_open("b", t - 2, 4)
                elif t % 4 == 3 and t < 99:
                    mu_n("b", t - 3, t - 1, False)
                elif t % 4 == 0 and t >= 4:
                    mu_n("b", t - 2, t, True)
                    mu_close("b", t // 4 - 1)
                if t == 98:
                    mu_open("b", 96, 4)
                elif t == 99:
                    mu_n("b", 96, 98, False)
                    mu_n("b", 98, 100, True)
                    mu_close("b", 24, final=True)

            # No DMA primers: the first instruction reading each DMA'd
            # region carries the extra wait (Tile emits a standalone wait,
            # satisfied by arrival); later readers are pruned by the
            # per-engine sem scoreboard.
            for t in range(T):
                if t < T - 1:
                    # Step 99 would only produce h_100, which nothing reads.
                    rec_step("a", t)
                    rec_step("b", t)
                mu_pieces(t)

    nc.compile()
    _NC_CACHE["nc"] = nc
    return nc


def _prepare_in_maps(inputs):
    folded = _fold_weights(inputs)
    x = np.asarray(inputs["x"], np.float32)

    def hot_cols(c, w):
        U, ub, M, N, V, vb = folded[c]
        blob = np.zeros((128, HOTB), np.float32)
        blob[:, 0:256] = U
        blob[0, 256:384] = ub[128:] - ub[:128]
        blob[0, 384:384 + w] = 1.0
        return blob

    def m_cols(c):
        U, ub, M, N, V, vb = folded[c]
        return np.concatenate([M[:128], M[128:]], axis=1)

    def vn_cols(c):
        U, ub, M, N, V, vb = folded[c]
        return np.concatenate([V, N[:128], N[128:]], axis=1)

    in_maps = []
    for core in range(8):
        ca, ba = PIECES_A[core]
        cb, bb = PIECES_B[core]
        xa = x[ca, :, ba:ba + WA, :].transpose(2, 0, 1)   # [128, T, WA]
        xb = x[cb, :, bb:bb + WB, :].transpose(2, 0, 1)   # [128, T, WB]
        xall = np.concatenate([xa, xb], axis=2)           # [128, T, 192]
        wbh = np.zeros((128, BHOT), np.float32)
        wbh[:, :2 * HOTB] = np.concatenate(
            [hot_cols(ca, WA), hot_cols(cb, WB)], axis=1)
        wm = np.concatenate([m_cols(ca), m_cols(cb)], axis=1)
        wvn = np.concatenate([vn_cols(ca), vn_cols(cb)], axis=1)
        wbh[:, 2 * HOTB] = folded[ca][1][:128]
        wbh[:, 2 * HOTB + 1] = folded[cb][1][:128]
        in_maps.append({
            "xT": np.ascontiguousarray(xall).astype(BF16),
            "wbh": wbh.astype(BF16),
            "wm": wm.astype(np.float32).astype(BF16),
            "wvn": wvn.astype(np.float32).astype(BF16),
        })
    return in_maps, folded


def kernel(**inputs):
    from concourse.bass_utils import run_bass_kernel_spmd

    in_maps, folded = _prepare_in_maps(inputs)
    nc = _build_nc()
    trace = bool(int(os.environ.get("BASS_KERNEL_TRACE", "0")))
    res = run_bass_kernel_spmd(nc, in_maps, core_ids=list(range(8)),
                               trace=trace)
    kernel.last_results = res

    out = np.empty((C, T, B, F), np.float32)
    for core in range(8):
        for sfx, w, (c, b0) in (("a", WA, PIECES_A[core]),
                                ("b", WB, PIECES_B[core])):
            muT = np.asarray(res.results[core][f"out_{sfx}"]).astype(np.float32)
            vb = folded[c][5].astype(np.float32)
            out[c, :, b0:b0 + w, :] = muT.reshape(128, T, w).transpose(1, 2, 0) + vb
    return out
